# revision 1
# baseline (speedup 1.0000x reference)
"""Bass/Trainium2 kernel for nn_BaselineAttention — loop-reps variant.

Same dataflow as the baseline kernel (DPx2 over batch, TPx4 over heads,
transposed activations, flash-style softmax via ones-column in V, LN1 stats
AllReduce, fc ReduceScatter, LN2 on scattered rows), but the timing reps are
a single `tc.For_i` hardware loop around one static copy of the body instead
of n_reps unrolled copies.  All APs in the body are static, so re-executions
run from the instruction cache instead of paying the per-instruction
first-execution cost this runtime charges.
"""

import contextlib

import numpy as np

import concourse.bacc as bacc
import concourse.mybir as mybir
import concourse.tile as tile
from concourse.bass_utils import run_bass_kernel_spmd

F32 = mybir.dt.float32
F32R = mybir.dt.float32r
AF = mybir.ActivationFunctionType
OP = mybir.AluOpType
AX = mybir.AxisListType

B, S, D, H = 2, 2048, 1024, 16
EPS = 1e-3
SCALE = 0.125            # 1/sqrt(D/H)
GROUPS = [[0, 1, 2, 3], [4, 5, 6, 7]]

_BUILD_CACHE = {}


def _build(n_reps=1):
    key = n_reps
    if key in _BUILD_CACHE:
        return _BUILD_CACHE[key]
    r1 = rar = r2 = rrs = r3 = n_reps

    nc = bacc.Bacc("TRN2", target_bir_lowering=False, debug=False, num_devices=8)

    xt_d = nc.dram_tensor("xt", [128, 8, S], F32R, kind="ExternalInput").ap()
    wq_d = nc.dram_tensor("wq", [128, 8, 128], F32R, kind="ExternalInput").ap()
    wk_d = nc.dram_tensor("wk", [128, 8, 128], F32R, kind="ExternalInput").ap()
    wv_d = nc.dram_tensor("wv", [128, 8, 256], F32R, kind="ExternalInput").ap()
    wfc_d = nc.dram_tensor("wfc", [128, 2, D], F32R, kind="ExternalInput").ap()
    onesrow_d = nc.dram_tensor("onesrow", [1, 128], F32R, kind="ExternalInput").ap()
    onesmat_d = nc.dram_tensor("onesmat", [128, 128], F32R, kind="ExternalInput").ap()
    onescol_d = nc.dram_tensor("onescol", [128, 1], F32R, kind="ExternalInput").ap()
    onesv_d = nc.dram_tensor("onesv", [128, 64], F32R, kind="ExternalInput").ap()
    ident_d = nc.dram_tensor("ident", [128, 128], F32R, kind="ExternalInput").ap()
    bq_d = nc.dram_tensor("bq", [128, 1], F32, kind="ExternalInput").ap()
    bk_d = nc.dram_tensor("bk", [128, 1], F32, kind="ExternalInput").ap()
    bv_d = nc.dram_tensor("bv", [128, 2], F32, kind="ExternalInput").ap()
    g1_d = nc.dram_tensor("g1", [128, 2], F32, kind="ExternalInput").ap()
    b1_d = nc.dram_tensor("b1", [128, 2], F32, kind="ExternalInput").ap()
    g2row_d = nc.dram_tensor("g2row", [1, D], F32R, kind="ExternalInput").ap()
    b2row_d = nc.dram_tensor("b2row", [1, D], F32R, kind="ExternalInput").ap()
    bfcrow_d = nc.dram_tensor("bfcrow", [1, D], F32R, kind="ExternalInput").ap()
    out_d = nc.dram_tensor("out", [4, 128, D], F32, kind="ExternalOutput").ap()

    with (
        tile.TileContext(nc) as tc,
        tc.tile_pool(name="sb", bufs=1) as sb,
        tc.tile_pool(name="ps", bufs=1, space="PSUM") as ps,
        tc.tile_pool(name="dr", bufs=1, space="DRAM") as dr,
    ):
        onesrow = sb.tile([1, 128], F32R)
        onesmat = sb.tile([128, 128], F32R)
        onescol = sb.tile([128, 1], F32R)
        ident = sb.tile([128, 128], F32R)
        wq = sb.tile([128, 8, 128], F32R)
        wk = sb.tile([128, 8, 128], F32R)
        wv = sb.tile([128, 8, 256], F32R)
        wfc = sb.tile([128, 2, D], F32R)
        bq = sb.tile([128, 1], F32)
        bk = sb.tile([128, 1], F32)
        bv = sb.tile([128, 2], F32)
        g1 = sb.tile([128, 2], F32)
        b1 = sb.tile([128, 2], F32)
        for t, d in [(onesrow, onesrow_d), (onesmat, onesmat_d),
                     (onescol, onescol_d), (ident, ident_d),
                     (wq, wq_d), (wk, wk_d), (wv, wv_d), (wfc, wfc_d),
                     (bq, bq_d), (bk, bk_d), (bv, bv_d), (g1, g1_d), (b1, b1_d)]:
            nc.sync.dma_start(t[:], d[:])

        # broadcast gamma2 / beta2 / bfc rows to [128, D] once (the [1, D]
        # staging row cycles one shared buffer — startup only)
        g2bc = sb.tile([128, D], F32)
        b2bc = sb.tile([128, D], F32)
        fcbc = sb.tile([128, D], F32)
        for row_d, dst in [(g2row_d, g2bc), (b2row_d, b2bc), (bfcrow_d, fcbc)]:
            row = sb.tile([1, D], F32R, tag="prow", bufs=1, name=f"row_{dst.name}")
            nc.sync.dma_start(row[:], row_d[:])
            bc_ps = ps.tile([128, 2048], F32, tag="tagA", bufs=1, name=f"bc_{dst.name}")
            for nch in range(2):
                nc.tensor.matmul(bc_ps[:, 512 * nch:512 * nch + 512],
                                 onesrow[:], row[0:1, 512 * nch:512 * nch + 512],
                                 start=True, stop=True)
            nc.vector.tensor_copy(dst[:], bc_ps[:, 0:1024])

        # persistent state
        vnat = sb.tile([128, 16, 260], F32R)   # V natural + ones cols
        nc.sync.dma_start(
            vnat[:].rearrange("p t (h x) -> p (t h) x", h=4)[:, :, 64:65],
            onesv_d[:].unsqueeze(2),
        )
        qt_sb = sb.tile([128, S], F32R)        # [qk-feat, tok]
        kt_sb = sb.tile([128, S], F32R)
        ysb_t = sb.tile([128, 2, S], F32R)     # [vfeat-local, jj, tok]
        ysb = ysb_t[:]
        ut_sb = ysb_t[:].bitcast(F32)          # f32 view for DVE reads
        yn_t = sb.tile([128, 2, S], F32R)      # LN1-normalized copy of ysb
        yn = yn_t[:]
        ynf = yn_t[:].bitcast(F32)

        stats_in = dr.tile([2, S], F32R)       # [sum; sumsq]
        stats_out = dr.tile([2, S], F32R)
        rs_in = dr.tile([S, D], F32)
        rs_out = dr.tile([512, D], F32)

        def loop(name, r):
            return (tc.For_i(0, r, 1, name=name) if r > 1
                    else contextlib.nullcontext())

        with loop("reploop1", r1):
            # ---------------- P1: projections ----------------
            for half in range(4):
                xs = sb.tile([128, 8, 512], F32R, tag="xs", bufs=1,
                             name=f"xs{half}")
                nc.sync.dma_start(xs[:], xt_d[:, :, 512 * half:512 * half + 512])
                for pname, w_t, mcol, bias, dst in [
                    ("q", wq, None, bq[:], qt_sb[:]),
                    ("k", wk, None, bk[:], kt_sb[:]),
                    ("v0", wv, slice(0, 128), bv[:, 0:1], None),
                    ("v1", wv, slice(128, 256), bv[:, 1:2], None),
                ]:
                    p_t = ps.tile([128, 512], F32,
                                  tag="tagA" if pname in ("q", "v0") else "tagB",
                                  bufs=1, name=f"p{pname}{half}")
                    for kc in range(8):
                        nc.tensor.matmul(
                            p_t[:],
                            w_t[:, kc, :] if mcol is None else w_t[:, kc, mcol],
                            xs[:, kc, :],
                            start=(kc == 0), stop=(kc == 7))
                    if pname in ("q", "k"):
                        nc.vector.tensor_scalar(
                            dst[:, 512 * half:512 * half + 512], p_t[:],
                            bias, None, OP.add)
                    else:
                        jj = 0 if pname == "v0" else 1
                        vt_st = sb.tile([128, 512], F32R, tag="vtst", bufs=1,
                                        name=f"vt{jj}{half}")
                        nc.vector.tensor_scalar(vt_st[:], p_t[:], bias, None, OP.add)
                        t_ps = ps.tile([128, 512], F32,
                                       tag="tagA" if pname == "v0" else "tagB",
                                       bufs=1, name=f"t{jj}{half}")
                        for blk in range(4):
                            nc.tensor.transpose(
                                t_ps[:, 128 * blk:128 * blk + 128].bitcast(F32R),
                                vt_st[:, 128 * blk:128 * blk + 128], ident[:])
                        nc.vector.tensor_copy(
                            vnat[:, 4 * half:4 * half + 4, :]
                            .rearrange("p t (h x) -> p t h x", h=4)
                            [:, :, 2 * jj:2 * jj + 2, 0:64],
                            t_ps[:].rearrange("p (t h x) -> p t h x", t=4, h=2),
                        )

            # ---------------- P2: attention (+ per-head d broadcast) --------
            dstages = []
            for jj in range(2):
                dstages.append(sb.tile([128, 2048], F32, tag="ln1t", bufs=3,
                                       name=f"dst{jj}"))
            for hl in range(4):
                s_ps = ps.tile([128, 2048], F32, tag="tagA", bufs=1,
                               name=f"s{hl}")
                u_ps = ps.tile([65, 2048], F32, tag="tagB", bufs=1,
                               name=f"u{hl}")
                e_t = sb.tile([128, 2048], F32R, tag="e", bufs=1, name=f"e{hl}")
                for kc in range(16):
                    for u in range(4):
                        nc.tensor.matmul(
                            s_ps[:, 512 * u:512 * u + 512],
                            kt_sb[32 * hl:32 * hl + 32, 128 * kc:128 * kc + 128],
                            qt_sb[32 * hl:32 * hl + 32, 512 * u:512 * u + 512],
                            tile_position=(32 * hl, 0), start=True, stop=True)
                    nc.scalar.activation(e_t[:], s_ps[:], AF.Exp, scale=SCALE)
                    for u in range(4):
                        nc.tensor.matmul(
                            u_ps[:, 512 * u:512 * u + 512],
                            vnat[:, kc, 65 * hl:65 * hl + 65],
                            e_t[:, 512 * u:512 * u + 512],
                            start=(kc == 0), stop=(kc == 15))
                u_st = sb.tile([65, S], F32R, tag="ust", bufs=2, name=f"ust{hl}")
                nc.vector.tensor_copy(u_st[:], u_ps[:])
                nc.sync.dma_start(
                    ysb[64 * (hl % 2):64 * (hl % 2) + 64, hl // 2, :],
                    u_st[0:64, :])
                # broadcast this head's denominator row (at partition 64)
                db = ps.tile([128, 2048], F32, tag="tagA", bufs=1,
                             name=f"db{hl}")
                for u in range(4):
                    nc.tensor.matmul(
                        db[:, 512 * u:512 * u + 512], onesmat[64:65, :],
                        u_st[64:65, 512 * u:512 * u + 512], start=True, stop=True)
                half = hl % 2
                nc.vector.tensor_copy(
                    dstages[hl // 2][64 * half:64 * half + 64, :],
                    db[64 * half:64 * half + 64, :])

            # ---------------- P3: divide, LN1 stats + AR, normalize ----------
            for jj in range(2):
                rec_t = sb.tile([128, 2048], F32, tag="ln1t", bufs=3,
                                name=f"rec{jj}")
                nc.vector.reciprocal_approx_fast(rec_t[:], dstages[jj][:])
                nc.vector.tensor_tensor(ysb[:, jj, :], ut_sb[:, jj, :], rec_t[:],
                                        OP.mult)
            st_s = ps.tile([1, 2048], F32, tag="tagA", bufs=1, name="sts")
            st_q = ps.tile([1, 2048], F32, tag="tagB", bufs=1, name="stq")
            for jj in range(2):
                ysq = sb.tile([128, 2048], F32R, tag="ln1t", bufs=3,
                              name=f"ysq{jj}")
                nc.vector.tensor_tensor(ysq[:], ysb[:, jj, :], ysb[:, jj, :], OP.mult)
                for u in range(4):
                    usl = slice(512 * u, 512 * u + 512)
                    nc.tensor.matmul(st_s[0:1, usl], onescol[:], ysb[:, jj, usl],
                                     start=(jj == 0), stop=(jj == 1))
                    nc.tensor.matmul(st_q[0:1, usl], onescol[:], ysq[:, usl],
                                     start=(jj == 0), stop=(jj == 1))
            ss_st = sb.tile([1, 2048], F32R, tag="row", bufs=2, name="ssst")
            sq_st = sb.tile([1, 2048], F32R, tag="row", bufs=2, name="sqst")
            nc.vector.tensor_copy(ss_st[:], st_s[0:1, :])
            nc.vector.tensor_copy(sq_st[:], st_q[0:1, :])
            nc.sync.dma_start(stats_in[0:1, :], ss_st[:])
            nc.sync.dma_start(stats_in[1:2, :], sq_st[:])

        # collectives cannot live inside a For_i loop on this runtime
        # (NRT_EXEC_UNIT_UNRECOVERABLE), so the timing reps run them
        # unrolled between the loops; every rep's data is identical, so
        # results are unchanged and each rep still executes them once.
        for _ in range(rar):
            nc.gpsimd.collective_compute(
                "AllReduce", OP.add, replica_groups=GROUPS,
                ins=[stats_in[:]], outs=[stats_out[:]])

        with loop("reploop2", r2):
            str_s = sb.tile([1, 2048], F32R, tag="row", bufs=2, name="strs")
            str_q = sb.tile([1, 2048], F32R, tag="row", bufs=2, name="strq")
            nc.sync.dma_start(str_s[:], stats_out[0:1, :])
            nc.sync.dma_start(str_q[:], stats_out[1:2, :])

            bs_ps = ps.tile([128, 2048], F32, tag="tagA", bufs=1, name="bs")
            bq_ps = ps.tile([128, 2048], F32, tag="tagB", bufs=1, name="bq2")
            for u in range(4):
                usl = slice(512 * u, 512 * u + 512)
                nc.tensor.matmul(bs_ps[:, usl], onesrow[:], str_s[0:1, usl],
                                 start=True, stop=True)
                nc.tensor.matmul(bq_ps[:, usl], onesrow[:], str_q[0:1, usl],
                                 start=True, stop=True)
            t_mu = sb.tile([128, 2048], F32, tag="ln1t", bufs=3, name="tmu")
            t_v = sb.tile([128, 2048], F32, tag="ln1t", bufs=3, name="tv")
            t_w = sb.tile([128, 2048], F32, tag="ln1t", bufs=3, name="tw")
            nc.vector.tensor_scalar(t_mu[:], bs_ps[:], 1.0 / D, None, OP.mult)
            nc.vector.tensor_scalar(t_v[:], bq_ps[:], 1.0 / D, None, OP.mult)
            nc.vector.tensor_tensor(t_w[:], t_mu[:], t_mu[:], OP.mult)
            nc.vector.tensor_tensor(t_v[:], t_v[:], t_w[:], OP.subtract)
            nc.vector.tensor_scalar(t_v[:], t_v[:], EPS, None, OP.add)
            nc.vector.reciprocal_approx_fast(t_w[:], t_v[:])
            nc.scalar.activation(t_v[:], t_w[:], AF.Sqrt)                 # r
            nc.vector.tensor_tensor(t_w[:], t_mu[:], t_v[:], OP.mult)     # mu*r
            for jj in range(2):
                nc.vector.tensor_tensor(yn[:, jj, :], ut_sb[:, jj, :], t_v[:], OP.mult)
                nc.vector.tensor_tensor(yn[:, jj, :], ynf[:, jj, :], t_w[:],
                                        OP.subtract)
                nc.vector.tensor_scalar(yn[:, jj, :], ynf[:, jj, :],
                                        g1[:, jj:jj + 1], b1[:, jj:jj + 1],
                                        OP.mult, OP.add)

            # ---------------- fc + RS ----------------
            for pair in range(8):       # 2 token-chunks of 128 per psum tile
                fc_ps = ps.tile([128, 2048], F32,
                                tag="tagA" if pair % 2 == 0 else "tagB",
                                bufs=1, name=f"fc{pair}")
                for half in range(2):
                    tok = slice(256 * pair + 128 * half, 256 * pair + 128 * half + 128)
                    for jj in range(2):
                        for nch in range(2):
                            nc.tensor.matmul(
                                fc_ps[:, 1024 * half + 512 * nch:
                                      1024 * half + 512 * nch + 512],
                                yn[:, jj, tok],
                                wfc[:, jj, 512 * nch:512 * nch + 512],
                                start=(jj == 0), stop=(jj == 1))
                p_st = sb.tile([128, 2048], F32, tag="pst", bufs=1,
                               name=f"pst{pair}")
                nc.vector.tensor_copy(p_st[:], fc_ps[:])
                nc.sync.dma_start(
                    rs_in[256 * pair:256 * pair + 256, :]
                    .rearrange("(t p) n -> p t n", t=2),
                    p_st[:].rearrange("p (t n) -> p t n", t=2))

        for _ in range(rrs):
            nc.gpsimd.collective_compute(
                "ReduceScatter", OP.add, replica_groups=GROUPS,
                ins=[rs_in[:]], outs=[rs_out[:]])

        # ---------------- LN2 ----------------
        with loop("reploop3", r3):
            for ts in range(4):
                pP = sb.tile([128, D], F32, tag="pP", bufs=1, name=f"pP{ts}")
                nc.sync.dma_start(pP[:], rs_out[128 * ts:128 * ts + 128, :])
                nc.vector.tensor_tensor(pP[:], pP[:], fcbc[:], OP.add)
                s2 = sb.tile([128, 8], F32, tag="s2", bufs=2, name=f"s2{ts}")
                nc.vector.tensor_reduce(s2[:, 0:1], pP[:], AX.X, OP.add)
                sqd = sb.tile([128, D], F32, tag="ln1t", bufs=3, name=f"sqd{ts}")
                nc.scalar.activation(sqd[:], pP[:], AF.Square, accum_out=s2[:, 1:2])
                nc.vector.tensor_scalar(s2[:, 0:1], s2[:, 0:1], 1.0 / D, None, OP.mult)
                nc.vector.tensor_scalar(s2[:, 1:2], s2[:, 1:2], 1.0 / D, None, OP.mult)
                nc.vector.tensor_tensor(s2[:, 2:3], s2[:, 0:1], s2[:, 0:1], OP.mult)
                nc.vector.tensor_tensor(s2[:, 3:4], s2[:, 1:2], s2[:, 2:3], OP.subtract)
                nc.vector.tensor_scalar(s2[:, 3:4], s2[:, 3:4], EPS, None, OP.add)
                nc.vector.reciprocal_approx_fast(s2[:, 4:5], s2[:, 3:4])
                nc.scalar.activation(s2[:, 5:6], s2[:, 4:5], AF.Sqrt)
                nc.vector.tensor_scalar(pP[:], pP[:], s2[:, 0:1], s2[:, 5:6],
                                        OP.subtract, OP.mult)
                nc.vector.tensor_tensor(pP[:], pP[:], g2bc[:], OP.mult)
                nc.vector.tensor_tensor(pP[:], pP[:], b2bc[:], OP.add)
                nc.sync.dma_start(out_d[ts], pP[:])

    nc.compile()
    _BUILD_CACHE[key] = nc
    return nc


def make_in_maps(x, Wq, bq, Wk, bk, Wv, bv, gamma1, beta1, Wfc, bfc, gamma2, beta2):
    x = np.asarray(x, np.float32)
    in_maps = []
    onesrow = np.ones((1, 128), np.float32)
    onesmat = np.ones((128, 128), np.float32)
    onescol = np.ones((128, 1), np.float32)
    onesv = np.ones((128, 64), np.float32)
    ident = np.eye(128, dtype=np.float32)
    Wq, Wk, Wv, Wfc = (np.asarray(a, np.float32) for a in (Wq, Wk, Wv, Wfc))
    for c in range(8):
        g, r = c // 4, c % 4
        xt = np.ascontiguousarray(
            x[g].T.reshape(8, 128, S).transpose(1, 0, 2))          # [128, 8, S]
        wq_c = np.ascontiguousarray(
            Wq[:, 128 * r:128 * r + 128].reshape(8, 128, 128).transpose(1, 0, 2))
        wk_c = np.ascontiguousarray(
            Wk[:, 128 * r:128 * r + 128].reshape(8, 128, 128).transpose(1, 0, 2))
        wv_c = np.ascontiguousarray(
            Wv[:, 256 * r:256 * r + 256].reshape(8, 128, 256).transpose(1, 0, 2))
        wfc_c = np.ascontiguousarray(
            Wfc[256 * r:256 * r + 256, :].reshape(2, 128, D).transpose(1, 0, 2))
        in_maps.append({
            "xt": xt, "wq": wq_c, "wk": wk_c, "wv": wv_c, "wfc": wfc_c,
            "onesrow": onesrow, "onesmat": onesmat, "onescol": onescol,
            "onesv": onesv, "ident": ident,
            "bq": np.asarray(bq, np.float32)[128 * r:128 * r + 128, None],
            "bk": np.asarray(bk, np.float32)[128 * r:128 * r + 128, None],
            "bv": np.asarray(bv, np.float32)[256 * r:256 * r + 256]
                 .reshape(2, 128).T.copy(),
            "g1": np.asarray(gamma1, np.float32)[256 * r:256 * r + 256]
                 .reshape(2, 128).T.copy(),
            "b1": np.asarray(beta1, np.float32)[256 * r:256 * r + 256]
                 .reshape(2, 128).T.copy(),
            "g2row": np.asarray(gamma2, np.float32)[None, :].copy(),
            "b2row": np.asarray(beta2, np.float32)[None, :].copy(),
            "bfcrow": np.asarray(bfc, np.float32)[None, :].copy(),
        })
    return in_maps


def assemble(results):
    out = np.empty((B, S, D), np.float32)
    for c in range(8):
        g, r = c // 4, c % 4
        o = results[c]["out"]                   # [4, 128, D] = slab r of batch g
        for ts in range(4):
            out[g, 512 * r + 128 * ts:512 * r + 128 * ts + 128, :] = o[ts]
    return out


def kernel(**inputs):
    nc = _build()
    in_maps = make_in_maps(**{k: np.asarray(v) for k, v in inputs.items()})
    res = run_bass_kernel_spmd(nc, in_maps, list(range(8)))
    return assemble(res.results)



# revision 4
# speedup vs baseline: 3.7140x; 3.7140x over previous
"""Bass/Trainium2 kernel for nn_BaselineAttention — zero-collective variant.

Sharding: pure DP over 8 output slabs of 512 tokens (core c -> batch c//4,
token rows 512*(c%4)..+512).  Each core recomputes K/V for its full batch
locally (collectives on this runtime cost ~160-330us fixed, far more than
the ~60us of recomputed PE work), so the whole forward pass lives in ONE
For_i hardware loop with no collectives and no cross-section barriers.

All matmuls run in bf16 (1 cycle/row on PE at any moving size); PSUM
accumulation stays f32.  Attention uses the flash-style ones-column in V to
get softmax denominators for free; activations stay feature-major
throughout so LayerNorm row-stats come from ones-matmuls.
"""

import contextlib

import ml_dtypes
import numpy as np

import concourse.bacc as bacc
import concourse.mybir as mybir
import concourse.tile as tile
from concourse.bass_utils import run_bass_kernel_spmd

F32 = mybir.dt.float32
BF16 = mybir.dt.bfloat16
AF = mybir.ActivationFunctionType
OP = mybir.AluOpType

B, S, D, H = 2, 2048, 1024, 16
EPS = 1e-3
SCALE = 0.125            # 1/sqrt(D/H)
NPBF = ml_dtypes.bfloat16

_BUILD_CACHE = {}


def _build(n_reps=1, phases="all"):
    if isinstance(n_reps, tuple):
        n_reps = max(n_reps)
    key = n_reps
    if key in _BUILD_CACHE:
        return _BUILD_CACHE[key]

    nc = bacc.Bacc("TRN2", target_bir_lowering=False, debug=False, num_devices=8)

    xt_d = nc.dram_tensor("xt", [128, 8, S], BF16, kind="ExternalInput").ap()
    xq_d = nc.dram_tensor("xq", [128, 8, 512], BF16, kind="ExternalInput").ap()
    wq_d = nc.dram_tensor("wq", [128, 8, 512], BF16, kind="ExternalInput").ap()
    wk_d = nc.dram_tensor("wk", [128, 8, 512], BF16, kind="ExternalInput").ap()
    wv_d = nc.dram_tensor("wv", [128, 8, 1024], BF16, kind="ExternalInput").ap()
    wfc_d = nc.dram_tensor("wfc", [128, 8, 1024], BF16, kind="ExternalInput").ap()
    bq_d = nc.dram_tensor("bq", [128, 4], F32, kind="ExternalInput").ap()
    bk_d = nc.dram_tensor("bk", [128, 4], F32, kind="ExternalInput").ap()
    bfc_d = nc.dram_tensor("bfc", [128, 8], F32, kind="ExternalInput").ap()
    g1_d = nc.dram_tensor("g1", [128, 8], F32, kind="ExternalInput").ap()
    b1_d = nc.dram_tensor("b1", [128, 8], F32, kind="ExternalInput").ap()
    g2_d = nc.dram_tensor("g2", [128, 8], F32, kind="ExternalInput").ap()
    b2_d = nc.dram_tensor("b2", [128, 8], F32, kind="ExternalInput").ap()
    bv_d = nc.dram_tensor("bvt", [128, 8], F32, kind="ExternalInput").ap()
    onescol_d = nc.dram_tensor("onescol", [128, 1], BF16, kind="ExternalInput").ap()
    invdcol_d = nc.dram_tensor("invdcol", [128, 1], BF16, kind="ExternalInput").ap()
    onesrow_d = nc.dram_tensor("onesrow", [1, 128], BF16, kind="ExternalInput").ap()
    onesv_d = nc.dram_tensor("onesv", [128, 256], BF16, kind="ExternalInput").ap()
    e64_d = nc.dram_tensor("e64", [1, 256], BF16, kind="ExternalInput").ap()
    out_d = nc.dram_tensor("out", [8, 128, 512], BF16, kind="ExternalOutput").ap()

    with (
        tile.TileContext(nc) as tc,
        tc.tile_pool(name="sb", bufs=1) as sb,
        tc.tile_pool(name="ps", bufs=1, space="PSUM") as ps,
    ):
        # ---------------- persistent weights / constants ----------------
        wq = sb.tile([128, 8, 512], BF16)
        wk = sb.tile([128, 8, 512], BF16)
        wv = sb.tile([128, 8, 1024], BF16)
        wfc = sb.tile([128, 8, 1024], BF16)
        bq = sb.tile([128, 4], F32)
        bk = sb.tile([128, 4], F32)
        bfc = sb.tile([128, 8], F32)
        g1 = sb.tile([128, 8], F32)
        b1 = sb.tile([128, 8], F32)
        g2 = sb.tile([128, 8], F32)
        b2 = sb.tile([128, 8], F32)
        onescol = sb.tile([128, 1], BF16)
        invdcol = sb.tile([128, 1], BF16)
        onesrow = sb.tile([1, 128], BF16)
        onesv = sb.tile([128, 256], BF16)
        e64 = sb.tile([1, 256], BF16)
        bvt = sb.tile([128, 8], F32)
        for t, d in [(wq, wq_d), (wk, wk_d), (wv, wv_d), (wfc, wfc_d),
                     (bq, bq_d), (bk, bk_d), (bfc, bfc_d),
                     (g1, g1_d), (b1, b1_d), (g2, g2_d), (b2, b2_d),
                     (onescol, onescol_d), (invdcol, invdcol_d),
                     (onesrow, onesrow_d),
                     (onesv, onesv_d), (e64, e64_d), (bvt, bv_d)]:
            nc.sync.dma_start(t[:], d[:])

        # persistent activations
        # qt_z: per-head q with the other heads' 96 partition rows ZERO, so
        # score matmuls contract over the full 128 partitions with no
        # tile_position (32-row tiled matmuls measured ~2.5x slower on PE)
        qt_z = sb.tile([128, 16, 512], BF16)   # [qk-dim(pad), head, qtok]
        kt = sb.tile([128, 4, 2048], BF16)     # [qk-dim, kcol-chunk, ktok]
        vnat = sb.tile([128, 16, 1040], BF16)  # [ktok, kc, head*65 (v64+one)]
        yraw = sb.tile([128, 8, 512], BF16)    # attn out (pre/post divide)
        yn = sb.tile([128, 8, 512], BF16)      # LN1 output
        fco = sb.tile([128, 8, 512], BF16)     # fc output (pre-LN2)

        # zero-init qt_z once; reps only overwrite each head's 32 live rows
        for h in range(16):
            nc.vector.tensor_scalar(qt_z[:, h, :], wq[:, 0, 0:512], 0.0,
                                    None, OP.mult)

        # vnat ones-columns (written once; attnV's 65th stationary column)
        nc.vector.tensor_copy(
            vnat[:].rearrange("p k (h x) -> p k h x", x=65)[:, :, :, 64],
            onesv[:].rearrange("p (k h) -> p k h", k=16))

        def loop(name, r):
            return (tc.For_i(0, r, 1, name=name) if r > 1
                    else contextlib.nullcontext())

        # persistent SBUF rows carrying LN1 stats (mu, E[x^2]) across the
        # loop-body boundary (PSUM accumulators are evacuated here at the
        # end of P2 so no PSUM slot is held across iterations)
        ls1_sb = sb.tile([1, 512], F32)
        lq1_sb = sb.tile([1, 512], F32)

        # prologue: iteration 0's P3 block runs on zeros (its out_d write is
        # overwritten by the epilogue)
        nc.vector.tensor_scalar(ls1_sb[:], wq[0:1, 0, 0:512], 0.0, None,
                                OP.mult)
        nc.vector.tensor_scalar(lq1_sb[:], wq[0:1, 0, 0:512], 0.0, None,
                                OP.mult)
        for j in range(8):
            nc.vector.tensor_scalar(yraw[:, j, :], wq[:, 0, 0:512], 0.0,
                                    None, OP.mult)

        def ln_finish(sum_src, sq_src, suffix):
            """sum_src holds mu, sq_src holds E[x^2] (1/D-scaled stats).
            Returns broadcast [128,512] bf16 tiles (mu, rsqrt(var+eps))."""
            row = sb.tile([1, 2, 512], F32, tag="row", bufs=2,
                          name=f"row{suffix}")
            rowb_mu = sb.tile([1, 512], BF16, tag="rowb", bufs=4,
                              name=f"rowbm{suffix}")
            rowb_r = sb.tile([1, 512], BF16, tag="rowb", bufs=4,
                             name=f"rowbr{suffix}")
            nc.vector.tensor_copy(rowb_mu[:], sum_src[:])
            nc.vector.tensor_tensor(row[:, 0, :], rowb_mu[:], rowb_mu[:],
                                    OP.mult)                     # mu^2
            nc.vector.tensor_tensor(row[:, 1, :], sq_src[:], row[:, 0, :],
                                    OP.subtract)                 # var
            nc.vector.tensor_scalar(row[:, 1, :], row[:, 1, :], EPS,
                                    None, OP.add)
            nc.vector.reciprocal_approx_fast(row[:, 0, :], row[:, 1, :])
            nc.scalar.activation(row[:, 1, :], row[:, 0, :], AF.Sqrt)  # r
            nc.vector.tensor_copy(rowb_r[:], row[:, 1, :])
            mu_ps = ps.tile([128, 512], F32, tag="pA", bufs=2,
                            name=f"mb{suffix}")
            nc.tensor.matmul(mu_ps[:], onesrow[:], rowb_mu[:],
                             start=True, stop=True)
            mubc = sb.tile([128, 512], BF16, tag="rbc", bufs=2,
                           name=f"mubc{suffix}")
            nc.vector.tensor_copy(mubc[:], mu_ps[:])
            r_ps = ps.tile([128, 512], F32, tag="pA", bufs=2,
                           name=f"rp{suffix}")
            nc.tensor.matmul(r_ps[:], onesrow[:], rowb_r[:],
                             start=True, stop=True)
            rbc2 = sb.tile([128, 512], BF16, tag="rbc", bufs=2,
                           name=f"rbc{suffix}")
            nc.vector.tensor_copy(rbc2[:], r_ps[:])
            return mubc, rbc2

        def ln1_apply(mubc1, rbc1, tag):
            for j in range(8):
                nc.vector.tensor_tensor(yn[:, j, :], yraw[:, j, :],
                                        mubc1[:], OP.subtract)
                nc.vector.tensor_tensor(yn[:, j, :], yn[:, j, :],
                                        rbc1[:], OP.mult)
                nc.vector.tensor_scalar(yn[:, j, :], yn[:, j, :],
                                        g1[:, j:j + 1], b1[:, j:j + 1],
                                        OP.mult, OP.add)

        def fc_block(tag):
            # fc with LN2 stats interleaved per output chunk
            ls2 = ps.tile([1, 512], F32, tag="pU", bufs=2, name=f"ls2{tag}")
            lq2 = ps.tile([1, 512], F32, tag="pU", bufs=2, name=f"lq2{tag}")
            for oc in range(8):
                fps = ps.tile([128, 512], F32, tag="pA", bufs=2,
                              name=f"fc{tag}{oc}")
                for f in range(8):
                    nc.tensor.matmul(fps[:],
                                     wfc[:, f, 128 * oc:128 * oc + 128],
                                     yn[:, f, :], start=(f == 0),
                                     stop=(f == 7))
                nc.scalar.activation(fco[:, oc, :], fps[:], AF.Identity,
                                      bias=bfc[:, oc:oc + 1])
                nc.tensor.matmul(ls2[:], invdcol[:], fco[:, oc, :],
                                 start=(oc == 0), stop=(oc == 7))
                sqt = sb.tile([128, 512], BF16, tag="sq", bufs=2,
                              name=f"sq2{tag}{oc}")
                nc.vector.tensor_tensor(sqt[:], fco[:, oc, :],
                                        fco[:, oc, :], OP.mult)
                nc.tensor.matmul(lq2[:], invdcol[:], sqt[:],
                                 start=(oc == 0), stop=(oc == 7))
            return ls2, lq2

        def ln2_out(ls2, lq2, tag):
            mubc2, rbc2b = ln_finish(ls2, lq2, f"2{tag}")
            for oc in range(8):
                o_t = sb.tile([128, 512], BF16, tag="otb", bufs=2,
                              name=f"o{tag}{oc}")
                nc.vector.tensor_tensor(o_t[:], fco[:, oc, :], mubc2[:],
                                        OP.subtract)
                nc.vector.tensor_tensor(o_t[:], o_t[:], rbc2b[:], OP.mult)
                nc.vector.tensor_scalar(o_t[:], o_t[:],
                                        g2[:, oc:oc + 1], b2[:, oc:oc + 1],
                                        OP.mult, OP.add)
                nc.sync.dma_start(out_d[oc], o_t[:])

        def p1_q():
            xq = sb.tile([128, 8, 512], BF16, tag="xs", bufs=2, name="xq")
            nc.sync.dma_start(xq[:], xq_d[:])
            for qc in range(4):
                qps = ps.tile([128, 512], F32, tag="pA", bufs=2,
                              name=f"q{qc}")
                for kc in range(8):
                    nc.tensor.matmul(qps[:],
                                     wq[:, kc, 128 * qc:128 * qc + 128],
                                     xq[:, kc, :], start=(kc == 0),
                                     stop=(kc == 7))
                for hh in range(4):
                    po = 32 * hh
                    nc.scalar.activation(
                        qt_z[po:po + 32, 4 * qc + hh, :], qps[po:po + 32, :],
                        AF.Identity, bias=bq[po:po + 32, qc:qc + 1])

        def p1_kv(tcc):
            xs = sb.tile([128, 8, 512], BF16, tag="xs", bufs=2,
                         name=f"xs{tcc}")
            nc.sync.dma_start(xs[:], xt_d[:, :, 512 * tcc:512 * tcc + 512])
            for kc4 in range(4):
                kps = ps.tile([128, 512], F32, tag="pA", bufs=2,
                              name=f"k{tcc}_{kc4}")
                for kc in range(8):
                    nc.tensor.matmul(
                        kps[:], wk[:, kc, 128 * kc4:128 * kc4 + 128],
                        xs[:, kc, :], start=(kc == 0), stop=(kc == 7))
                nc.scalar.activation(
                    kt[:, kc4, 512 * tcc:512 * tcc + 512], kps[:],
                    AF.Identity, bias=bk[:, kc4:kc4 + 1])
            for sub in range(4):
                vps = ps.tile([128, 1024], F32, tag="pS", bufs=2,
                              name=f"v{tcc}_{sub}")
                for kc in range(8):
                    st = xs[:, kc, 128 * sub:128 * sub + 128]
                    nc.tensor.matmul(vps[:, 0:512], st, wv[:, kc, 0:512],
                                     start=(kc == 0), stop=(kc == 7))
                    nc.tensor.matmul(vps[:, 512:1024], st,
                                     wv[:, kc, 512:1024],
                                     start=(kc == 0), stop=(kc == 7))
                kci = 4 * tcc + sub
                vdst = vnat[:, kci, :].rearrange("p (h x) -> p h x", x=65)
                nc.scalar.activation(
                    vdst[:, :, 0:64],
                    vps[:].rearrange("p (h x) -> p h x", x=64), AF.Copy)

        def p2_attention():
            dens = {}
            ls1 = ps.tile([1, 512], F32, tag="pA", bufs=2, name="ls1")
            lq1 = ps.tile([1, 512], F32, tag="pA", bufs=2, name="lq1")

            def divide(j):
                rbp = ps.tile([128, 512], F32, tag="pU", bufs=2,
                              name=f"rb{j}")
                nc.tensor.matmul(rbp[:], e64[0:1, 0:128],
                                 dens[2 * j][:], start=True, stop=False)
                nc.tensor.matmul(rbp[:], e64[0:1, 128:256],
                                 dens[2 * j + 1][:], start=False, stop=True)
                rbf = sb.tile([128, 512], F32, tag="ot", bufs=2,
                              name=f"rbf{j}")
                nc.vector.reciprocal_approx_fast(rbf[:], rbp[:])
                rbc = sb.tile([128, 512], BF16, tag="rbc", bufs=2,
                              name=f"rbc{j}")
                nc.vector.tensor_copy(rbc[:], rbf[:])
                nc.vector.tensor_tensor(yraw[:, j, :], yraw[:, j, :],
                                        rbc[:], OP.mult)
                nc.vector.tensor_scalar(yraw[:, j, :], yraw[:, j, :],
                                        bvt[:, j:j + 1], None, OP.add)

            def ln1_stats(j):
                nc.tensor.matmul(ls1[:], invdcol[:], yraw[:, j, :],
                                 start=(j == 0), stop=(j == 7))
                sqt = sb.tile([128, 512], BF16, tag="sq", bufs=2,
                              name=f"sq1_{j}")
                nc.vector.tensor_tensor(sqt[:], yraw[:, j, :],
                                        yraw[:, j, :], OP.mult)
                nc.tensor.matmul(lq1[:], invdcol[:], sqt[:],
                                 start=(j == 0), stop=(j == 7))

            ups_t = {}

            def emit_u(ph, pk, pe):
                for half in range(2):
                    kc = 2 * pk + half
                    nc.tensor.matmul(
                        ups_t[ph][:], vnat[:, kc, 65 * ph:65 * ph + 65],
                        pe[:, 512 * half:512 * half + 512],
                        start=(kc == 0), stop=(kc == 15))

            def head_done(ph):
                # evacuate head ph; schedule lagged division / LN1 stats
                nc.vector.tensor_copy(
                    yraw[64 * (ph % 2):64 * (ph % 2) + 64, ph // 2, :],
                    ups_t[ph][0:64, :])
                den = sb.tile([1, 512], BF16, tag="den", bufs=6,
                              name=f"den{ph}")
                nc.vector.tensor_copy(den[:], ups_t[ph][64:65, :])
                dens[ph] = den
                if ph % 2 == 1 and ph >= 3:
                    jj = (ph - 1) // 2 - 1
                    divide(jj)
                    if jj >= 1:
                        ln1_stats(jj - 1)

            pend = []
            for h in range(16):
                ch = h // 4
                for k2 in range(8):
                    if k2 == 0:
                        ups_t[h] = ps.tile([65, 512], F32, tag="pU", bufs=2,
                                           name=f"u{h}")
                    sps = ps.tile([128, 1024], F32, tag="pS", bufs=2,
                                  name=f"s{h}_{k2}")
                    for half in range(2):
                        kc = 2 * k2 + half
                        nc.tensor.matmul(
                            sps[:, 512 * half:512 * half + 512],
                            kt[:, ch, 128 * kc:128 * kc + 128],
                            qt_z[:, h, :], start=True, stop=True)
                    et = sb.tile([128, 1024], BF16, tag="e", bufs=2,
                                 name=f"e{h}_{k2}")
                    nc.scalar.activation(et[:], sps[:], AF.Exp, scale=SCALE)
                    pend.append((h, k2, et))
                    if len(pend) > 1:
                        ph, pk, pe = pend.pop(0)
                        emit_u(ph, pk, pe)
                        if pk == 7:
                            head_done(ph)
            for ph, pk, pe in pend:
                emit_u(ph, pk, pe)
                if pk == 7:
                    head_done(ph)
            divide(7)
            ln1_stats(6)
            ln1_stats(7)
            # evacuate stats to SBUF so no PSUM slot crosses the iteration
            nc.vector.tensor_copy(ls1_sb[:], ls1[:])
            nc.vector.tensor_copy(lq1_sb[:], lq1[:])

        with loop("rep", n_reps):
            # P3(prev) interleaved with P1(cur): the in-order PE works on
            # projection matmuls while the DVE/Act run P3's LN chains
            mubc1, rbc1 = ln_finish(ls1_sb, lq1_sb, "1")
            p1_q()
            ln1_apply(mubc1, rbc1, "m")
            p1_kv(0)
            ls2, lq2 = fc_block("m")
            p1_kv(1)
            ln2_out(ls2, lq2, "m")
            p1_kv(2)
            p1_kv(3)
            p2_attention()

        # epilogue: P3 of the final rep
        mubc1, rbc1 = ln_finish(ls1_sb, lq1_sb, "1e")
        ln1_apply(mubc1, rbc1, "e")
        ls2, lq2 = fc_block("e")
        ln2_out(ls2, lq2, "e")

    nc.compile()
    _BUILD_CACHE[key] = nc
    return nc


def make_in_maps(x, Wq, bq, Wk, bk, Wv, bv, gamma1, beta1, Wfc, bfc, gamma2,
                 beta2):
    x = np.asarray(x, np.float32)
    Wq, Wk, Wv, Wfc = (np.asarray(a, np.float32) for a in (Wq, Wk, Wv, Wfc))
    wq_t = np.ascontiguousarray(
        Wq.reshape(8, 128, 512).transpose(1, 0, 2)).astype(NPBF)
    wk_t = np.ascontiguousarray(
        Wk.reshape(8, 128, 512).transpose(1, 0, 2)).astype(NPBF)
    wv_t = np.ascontiguousarray(
        Wv.reshape(8, 128, 1024).transpose(1, 0, 2)).astype(NPBF)
    wfc_t = np.ascontiguousarray(
        Wfc.reshape(8, 128, 1024).transpose(1, 0, 2)).astype(NPBF)
    bq_t = np.asarray(bq, np.float32).reshape(4, 128).T.copy()
    bk_t = np.asarray(bk, np.float32).reshape(4, 128).T.copy()
    bfc_t = np.asarray(bfc, np.float32).reshape(8, 128).T.copy()
    g1_t = np.asarray(gamma1, np.float32).reshape(8, 128).T.copy()
    b1_t = np.asarray(beta1, np.float32).reshape(8, 128).T.copy()
    g2_t = np.asarray(gamma2, np.float32).reshape(8, 128).T.copy()
    b2_t = np.asarray(beta2, np.float32).reshape(8, 128).T.copy()
    bvt = np.asarray(bv, np.float32).reshape(8, 128).T.copy()
    onescol = np.ones((128, 1), NPBF)
    invdcol = np.full((128, 1), 1.0 / D, NPBF)
    onesrow = np.ones((1, 128), NPBF)
    onesv = np.ones((128, 256), NPBF)
    e64 = np.zeros((1, 256), np.float32)
    e64[0, 0:64] = 1.0        # e64lo: broadcast to partitions 0-63
    e64[0, 192:256] = 1.0     # e64hi: broadcast to partitions 64-127
    e64 = e64.astype(NPBF)

    in_maps = []
    for c in range(8):
        g, r = c // 4, c % 4
        xt = np.ascontiguousarray(
            x[g].T.reshape(8, 128, S).transpose(1, 0, 2)).astype(NPBF)
        in_maps.append({
            "xt": xt,
            "xq": np.ascontiguousarray(xt[:, :, 512 * r:512 * r + 512]),
            "wq": wq_t, "wk": wk_t, "wv": wv_t, "wfc": wfc_t,
            "bq": bq_t, "bk": bk_t, "bfc": bfc_t,
            "g1": g1_t, "b1": b1_t, "g2": g2_t, "b2": b2_t,
            "bvt": bvt, "onescol": onescol, "invdcol": invdcol,
            "onesrow": onesrow,
            "onesv": onesv, "e64": e64,
        })
    return in_maps


def assemble(results):
    out = np.empty((B, S, D), np.float32)
    for c in range(8):
        g, r = c // 4, c % 4
        o = np.asarray(results[c]["out"], np.float32)   # [8, 128, 512]
        for j in range(8):
            out[g, 512 * r:512 * r + 512, 128 * j:128 * j + 128] = o[j].T
    return out


def kernel(**inputs):
    nc = _build()
    in_maps = make_in_maps(**{k: np.asarray(v) for k, v in inputs.items()})
    res = run_bass_kernel_spmd(nc, in_maps, list(range(8)))
    return assemble(res.results)


# revision 5
# speedup vs baseline: 3.7995x; 1.0230x over previous
"""Bass/Trainium2 kernel for nn_BaselineAttention — zero-collective variant.

Sharding: pure DP over 8 output slabs of 512 tokens (core c -> batch c//4,
token rows 512*(c%4)..+512).  Each core recomputes K/V for its full batch
locally (collectives on this runtime cost ~160-330us fixed, far more than
the ~60us of recomputed PE work), so the whole forward pass lives in ONE
For_i hardware loop with no collectives and no cross-section barriers.

All matmuls run in bf16 (1 cycle/row on PE at any moving size); PSUM
accumulation stays f32.  Attention uses the flash-style ones-column in V to
get softmax denominators for free; activations stay feature-major
throughout so LayerNorm row-stats come from ones-matmuls.
"""

import contextlib

import ml_dtypes
import numpy as np

import concourse.bacc as bacc
import concourse.mybir as mybir
import concourse.tile as tile
from concourse.bass_utils import run_bass_kernel_spmd

F32 = mybir.dt.float32
BF16 = mybir.dt.bfloat16
AF = mybir.ActivationFunctionType
OP = mybir.AluOpType

B, S, D, H = 2, 2048, 1024, 16
EPS = 1e-3
SCALE = 0.125            # 1/sqrt(D/H)
NPBF = ml_dtypes.bfloat16

_BUILD_CACHE = {}


def _build(n_reps=1, phases="all"):
    if isinstance(n_reps, tuple):
        n_reps = max(n_reps)
    key = n_reps
    if key in _BUILD_CACHE:
        return _BUILD_CACHE[key]

    nc = bacc.Bacc("TRN2", target_bir_lowering=False, debug=False, num_devices=8)

    xt_d = nc.dram_tensor("xt", [128, 8, S], BF16, kind="ExternalInput").ap()
    xq_d = nc.dram_tensor("xq", [128, 8, 512], BF16, kind="ExternalInput").ap()
    wq_d = nc.dram_tensor("wq", [128, 8, 512], BF16, kind="ExternalInput").ap()
    wk_d = nc.dram_tensor("wk", [128, 8, 512], BF16, kind="ExternalInput").ap()
    wv_d = nc.dram_tensor("wv", [128, 8, 1024], BF16, kind="ExternalInput").ap()
    wfc_d = nc.dram_tensor("wfc", [128, 8, 1024], BF16, kind="ExternalInput").ap()
    bq_d = nc.dram_tensor("bq", [128, 4], F32, kind="ExternalInput").ap()
    bk_d = nc.dram_tensor("bk", [128, 4], F32, kind="ExternalInput").ap()
    bfc_d = nc.dram_tensor("bfc", [128, 8], F32, kind="ExternalInput").ap()
    g1_d = nc.dram_tensor("g1", [128, 8], F32, kind="ExternalInput").ap()
    b1_d = nc.dram_tensor("b1", [128, 8], F32, kind="ExternalInput").ap()
    g2_d = nc.dram_tensor("g2", [128, 8], F32, kind="ExternalInput").ap()
    b2_d = nc.dram_tensor("b2", [128, 8], F32, kind="ExternalInput").ap()
    bv_d = nc.dram_tensor("bvt", [128, 8], F32, kind="ExternalInput").ap()
    onescol_d = nc.dram_tensor("onescol", [128, 1], BF16, kind="ExternalInput").ap()
    invdcol_d = nc.dram_tensor("invdcol", [128, 1], BF16, kind="ExternalInput").ap()
    onesrow_d = nc.dram_tensor("onesrow", [1, 128], BF16, kind="ExternalInput").ap()
    onesv_d = nc.dram_tensor("onesv", [128, 256], BF16, kind="ExternalInput").ap()
    e64_d = nc.dram_tensor("e64", [1, 256], BF16, kind="ExternalInput").ap()
    out_d = nc.dram_tensor("out", [8, 128, 512], BF16, kind="ExternalOutput").ap()

    with (
        tile.TileContext(nc) as tc,
        tc.tile_pool(name="sb", bufs=1) as sb,
        tc.tile_pool(name="ps", bufs=1, space="PSUM") as ps,
    ):
        # ---------------- persistent weights / constants ----------------
        wq = sb.tile([128, 8, 512], BF16)
        wk = sb.tile([128, 8, 512], BF16)
        wv = sb.tile([128, 8, 1024], BF16)
        wfc = sb.tile([128, 8, 1024], BF16)
        bq = sb.tile([128, 4], F32)
        bk = sb.tile([128, 4], F32)
        bfc = sb.tile([128, 8], F32)
        g1 = sb.tile([128, 8], F32)
        b1 = sb.tile([128, 8], F32)
        g2 = sb.tile([128, 8], F32)
        b2 = sb.tile([128, 8], F32)
        onescol = sb.tile([128, 1], BF16)
        invdcol = sb.tile([128, 1], BF16)
        onesrow = sb.tile([1, 128], BF16)
        onesv = sb.tile([128, 256], BF16)
        e64 = sb.tile([1, 256], BF16)
        bvt = sb.tile([128, 8], F32)
        for t, d in [(wq, wq_d), (wk, wk_d), (wv, wv_d), (wfc, wfc_d),
                     (bq, bq_d), (bk, bk_d), (bfc, bfc_d),
                     (g1, g1_d), (b1, b1_d), (g2, g2_d), (b2, b2_d),
                     (onescol, onescol_d), (invdcol, invdcol_d),
                     (onesrow, onesrow_d),
                     (onesv, onesv_d), (e64, e64_d), (bvt, bv_d)]:
            nc.sync.dma_start(t[:], d[:])

        # persistent activations
        # qt_z: per-head q with the other heads' 96 partition rows ZERO, so
        # score matmuls contract over the full 128 partitions with no
        # tile_position (32-row tiled matmuls measured ~2.5x slower on PE)
        qt_z = sb.tile([128, 16, 512], BF16)   # [qk-dim(pad), head, qtok]
        kt = sb.tile([128, 4, 2048], BF16)     # [qk-dim, kcol-chunk, ktok]
        vnat = sb.tile([128, 16, 1040], BF16)  # [ktok, kc, head*65 (v64+one)]
        yraw = sb.tile([128, 8, 512], BF16)    # attn out (pre/post divide)
        yn = sb.tile([128, 8, 512], BF16)      # LN1 output
        fco = sb.tile([128, 8, 512], BF16)     # fc output (pre-LN2)

        # zero-init qt_z once; reps only overwrite each head's 32 live rows
        for h in range(16):
            nc.vector.tensor_scalar(qt_z[:, h, :], wq[:, 0, 0:512], 0.0,
                                    None, OP.mult)

        # vnat ones-columns (written once; attnV's 65th stationary column)
        nc.vector.tensor_copy(
            vnat[:].rearrange("p k (h x) -> p k h x", x=65)[:, :, :, 64],
            onesv[:].rearrange("p (k h) -> p k h", k=16))

        def loop(name, r):
            return (tc.For_i(0, r, 1, name=name) if r > 1
                    else contextlib.nullcontext())

        # persistent SBUF rows carrying LN1 stats (mu, E[x^2]) across the
        # loop-body boundary (PSUM accumulators are evacuated here at the
        # end of P2 so no PSUM slot is held across iterations)
        ls1_sb = sb.tile([1, 512], F32)
        lq1_sb = sb.tile([1, 512], F32)

        # prologue: iteration 0's P3 block runs on zeros (its out_d write is
        # overwritten by the epilogue)
        nc.vector.tensor_scalar(ls1_sb[:], wq[0:1, 0, 0:512], 0.0, None,
                                OP.mult)
        nc.vector.tensor_scalar(lq1_sb[:], wq[0:1, 0, 0:512], 0.0, None,
                                OP.mult)
        for j in range(8):
            nc.vector.tensor_scalar(yraw[:, j, :], wq[:, 0, 0:512], 0.0,
                                    None, OP.mult)

        def ln_finish(sum_src, sq_src, suffix):
            """sum_src holds mu, sq_src holds E[x^2] (1/D-scaled stats).
            Returns broadcast [128,512] bf16 tiles (mu, rsqrt(var+eps))."""
            row = sb.tile([1, 2, 512], F32, tag="row", bufs=2,
                          name=f"row{suffix}")
            rowb_mu = sb.tile([1, 512], BF16, tag="rowb", bufs=4,
                              name=f"rowbm{suffix}")
            rowb_r = sb.tile([1, 512], BF16, tag="rowb", bufs=4,
                             name=f"rowbr{suffix}")
            nc.vector.tensor_copy(rowb_mu[:], sum_src[:])
            nc.vector.tensor_tensor(row[:, 0, :], rowb_mu[:], rowb_mu[:],
                                    OP.mult)                     # mu^2
            nc.vector.tensor_tensor(row[:, 1, :], sq_src[:], row[:, 0, :],
                                    OP.subtract)                 # var
            nc.vector.tensor_scalar(row[:, 1, :], row[:, 1, :], EPS,
                                    None, OP.add)
            nc.vector.reciprocal_approx_fast(row[:, 0, :], row[:, 1, :])
            nc.scalar.activation(row[:, 1, :], row[:, 0, :], AF.Sqrt)  # r
            nc.vector.tensor_copy(rowb_r[:], row[:, 1, :])
            mu_ps = ps.tile([128, 512], F32, tag="pA", bufs=2,
                            name=f"mb{suffix}")
            nc.tensor.matmul(mu_ps[:], onesrow[:], rowb_mu[:],
                             start=True, stop=True)
            mubc = sb.tile([128, 512], BF16, tag="rbc", bufs=2,
                           name=f"mubc{suffix}")
            nc.vector.tensor_copy(mubc[:], mu_ps[:])
            r_ps = ps.tile([128, 512], F32, tag="pA", bufs=2,
                           name=f"rp{suffix}")
            nc.tensor.matmul(r_ps[:], onesrow[:], rowb_r[:],
                             start=True, stop=True)
            rbc2 = sb.tile([128, 512], BF16, tag="rbc", bufs=2,
                           name=f"rbc{suffix}")
            nc.vector.tensor_copy(rbc2[:], r_ps[:])
            return mubc, rbc2

        def ln1_apply(mubc1, rbc1, tag):
            for j in range(8):
                nc.vector.tensor_tensor(yn[:, j, :], yraw[:, j, :],
                                        mubc1[:], OP.subtract)
                nc.vector.tensor_tensor(yn[:, j, :], yn[:, j, :],
                                        rbc1[:], OP.mult)
                nc.vector.tensor_scalar(yn[:, j, :], yn[:, j, :],
                                        g1[:, j:j + 1], b1[:, j:j + 1],
                                        OP.mult, OP.add)

        def fc_block(tag):
            # fc with LN2 stats interleaved per output chunk
            ls2 = ps.tile([1, 512], F32, tag="pU", bufs=2, name=f"ls2{tag}")
            lq2 = ps.tile([1, 512], F32, tag="pU", bufs=2, name=f"lq2{tag}")
            for oc in range(8):
                fps = ps.tile([128, 512], F32, tag="pA", bufs=2,
                              name=f"fc{tag}{oc}")
                for f in range(8):
                    nc.tensor.matmul(fps[:],
                                     wfc[:, f, 128 * oc:128 * oc + 128],
                                     yn[:, f, :], start=(f == 0),
                                     stop=(f == 7))
                nc.scalar.activation(fco[:, oc, :], fps[:], AF.Identity,
                                      bias=bfc[:, oc:oc + 1])
                nc.tensor.matmul(ls2[:], invdcol[:], fco[:, oc, :],
                                 start=(oc == 0), stop=(oc == 7))
                sqt = sb.tile([128, 512], BF16, tag="sq", bufs=2,
                              name=f"sq2{tag}{oc}")
                nc.vector.tensor_tensor(sqt[:], fco[:, oc, :],
                                        fco[:, oc, :], OP.mult)
                nc.tensor.matmul(lq2[:], invdcol[:], sqt[:],
                                 start=(oc == 0), stop=(oc == 7))
            return ls2, lq2

        def ln2_out(ls2, lq2, tag):
            mubc2, rbc2b = ln_finish(ls2, lq2, f"2{tag}")
            for oc in range(8):
                o_t = sb.tile([128, 512], BF16, tag="otb", bufs=2,
                              name=f"o{tag}{oc}")
                nc.vector.tensor_tensor(o_t[:], fco[:, oc, :], mubc2[:],
                                        OP.subtract)
                nc.vector.tensor_tensor(o_t[:], o_t[:], rbc2b[:], OP.mult)
                nc.vector.tensor_scalar(o_t[:], o_t[:],
                                        g2[:, oc:oc + 1], b2[:, oc:oc + 1],
                                        OP.mult, OP.add)
                nc.sync.dma_start(out_d[oc], o_t[:])

        def p1_q():
            xq = sb.tile([128, 8, 512], BF16, tag="xs", bufs=2, name="xq")
            nc.sync.dma_start(xq[:], xq_d[:])
            for qc in range(4):
                qps = ps.tile([128, 512], F32, tag="pA", bufs=2,
                              name=f"q{qc}")
                for kc in range(8):
                    nc.tensor.matmul(qps[:],
                                     wq[:, kc, 128 * qc:128 * qc + 128],
                                     xq[:, kc, :], start=(kc == 0),
                                     stop=(kc == 7))
                for hh in range(4):
                    po = 32 * hh
                    nc.scalar.activation(
                        qt_z[po:po + 32, 4 * qc + hh, :], qps[po:po + 32, :],
                        AF.Identity, bias=bq[po:po + 32, qc:qc + 1])

        def p1_kv(tcc):
            xs = sb.tile([128, 8, 512], BF16, tag="xs", bufs=2,
                         name=f"xs{tcc}")
            nc.sync.dma_start(xs[:], xt_d[:, :, 512 * tcc:512 * tcc + 512])
            for kc4 in range(4):
                kps = ps.tile([128, 512], F32, tag="pA", bufs=2,
                              name=f"k{tcc}_{kc4}")
                for kc in range(8):
                    nc.tensor.matmul(
                        kps[:], wk[:, kc, 128 * kc4:128 * kc4 + 128],
                        xs[:, kc, :], start=(kc == 0), stop=(kc == 7))
                nc.scalar.activation(
                    kt[:, kc4, 512 * tcc:512 * tcc + 512], kps[:],
                    AF.Identity, bias=bk[:, kc4:kc4 + 1])
            for sub in range(4):
                vps = ps.tile([128, 1024], F32, tag="pS", bufs=2,
                              name=f"v{tcc}_{sub}")
                for kc in range(8):
                    st = xs[:, kc, 128 * sub:128 * sub + 128]
                    nc.tensor.matmul(vps[:, 0:512], st, wv[:, kc, 0:512],
                                     start=(kc == 0), stop=(kc == 7))
                    nc.tensor.matmul(vps[:, 512:1024], st,
                                     wv[:, kc, 512:1024],
                                     start=(kc == 0), stop=(kc == 7))
                kci = 4 * tcc + sub
                vdst = vnat[:, kci, :].rearrange("p (h x) -> p h x", x=65)
                nc.scalar.activation(
                    vdst[:, :, 0:64],
                    vps[:].rearrange("p (h x) -> p h x", x=64), AF.Copy)

        def p2_attention():
            dens = {}
            ls1 = ps.tile([1, 512], F32, tag="pA", bufs=2, name="ls1")
            lq1 = ps.tile([1, 512], F32, tag="pA", bufs=2, name="lq1")

            def divide(j):
                rbp = ps.tile([128, 512], F32, tag="pU", bufs=2,
                              name=f"rb{j}")
                nc.tensor.matmul(rbp[:], e64[0:1, 0:128],
                                 dens[2 * j][:], start=True, stop=False)
                nc.tensor.matmul(rbp[:], e64[0:1, 128:256],
                                 dens[2 * j + 1][:], start=False, stop=True)
                rbf = sb.tile([128, 512], F32, tag="ot", bufs=2,
                              name=f"rbf{j}")
                nc.vector.reciprocal_approx_fast(rbf[:], rbp[:])
                rbc = sb.tile([128, 512], BF16, tag="rbc", bufs=2,
                              name=f"rbc{j}")
                nc.vector.tensor_copy(rbc[:], rbf[:])
                nc.vector.tensor_tensor(yraw[:, j, :], yraw[:, j, :],
                                        rbc[:], OP.mult)
                nc.vector.tensor_scalar(yraw[:, j, :], yraw[:, j, :],
                                        bvt[:, j:j + 1], None, OP.add)

            def ln1_stats(j):
                nc.tensor.matmul(ls1[:], invdcol[:], yraw[:, j, :],
                                 start=(j == 0), stop=(j == 7))
                sqt = sb.tile([128, 512], BF16, tag="sq", bufs=2,
                              name=f"sq1_{j}")
                nc.vector.tensor_tensor(sqt[:], yraw[:, j, :],
                                        yraw[:, j, :], OP.mult)
                nc.tensor.matmul(lq1[:], invdcol[:], sqt[:],
                                 start=(j == 0), stop=(j == 7))

            ups_t = {}

            def emit_u(ph, pk, pe):
                for half in range(2):
                    kc = 2 * pk + half
                    nc.tensor.matmul(
                        ups_t[ph][:], vnat[:, kc, 65 * ph:65 * ph + 65],
                        pe[:, 512 * half:512 * half + 512],
                        start=(kc == 0), stop=(kc == 15))

            def head_done(ph):
                # evacuate head ph; schedule lagged division / LN1 stats
                nc.vector.tensor_copy(
                    yraw[64 * (ph % 2):64 * (ph % 2) + 64, ph // 2, :],
                    ups_t[ph][0:64, :])
                den = sb.tile([1, 512], BF16, tag="den", bufs=6,
                              name=f"den{ph}")
                nc.vector.tensor_copy(den[:], ups_t[ph][64:65, :])
                dens[ph] = den
                if ph % 2 == 1 and ph >= 3:
                    jj = (ph - 1) // 2 - 1
                    divide(jj)
                    if jj >= 1:
                        ln1_stats(jj - 1)

            pend = []
            for h in range(16):
                ch = h // 4
                for k2 in range(8):
                    if k2 == 0:
                        ups_t[h] = ps.tile([65, 512], F32, tag="pU", bufs=2,
                                           name=f"u{h}")
                    sps = ps.tile([128, 1024], F32, tag="pS", bufs=2,
                                  name=f"s{h}_{k2}")
                    for half in range(2):
                        kc = 2 * k2 + half
                        nc.tensor.matmul(
                            sps[:, 512 * half:512 * half + 512],
                            kt[:, ch, 128 * kc:128 * kc + 128],
                            qt_z[:, h, :], start=True, stop=True)
                    et = sb.tile([128, 1024], BF16, tag="e", bufs=3,
                                 name=f"e{h}_{k2}")
                    nc.scalar.activation(et[:], sps[:], AF.Exp, scale=SCALE)
                    pend.append((h, k2, et))
                    if len(pend) > 2:
                        ph, pk, pe = pend.pop(0)
                        emit_u(ph, pk, pe)
                        if pk == 7:
                            head_done(ph)
            for ph, pk, pe in pend:
                emit_u(ph, pk, pe)
                if pk == 7:
                    head_done(ph)
            divide(7)
            ln1_stats(6)
            ln1_stats(7)
            # evacuate stats to SBUF so no PSUM slot crosses the iteration
            nc.vector.tensor_copy(ls1_sb[:], ls1[:])
            nc.vector.tensor_copy(lq1_sb[:], lq1[:])

        with loop("rep", n_reps):
            # P3(prev) interleaved with P1(cur): the in-order PE works on
            # projection matmuls while the DVE/Act run P3's LN chains
            mubc1, rbc1 = ln_finish(ls1_sb, lq1_sb, "1")
            p1_q()
            ln1_apply(mubc1, rbc1, "m")
            p1_kv(0)
            ls2, lq2 = fc_block("m")
            p1_kv(1)
            ln2_out(ls2, lq2, "m")
            p1_kv(2)
            p1_kv(3)
            p2_attention()

        # epilogue: P3 of the final rep
        mubc1, rbc1 = ln_finish(ls1_sb, lq1_sb, "1e")
        ln1_apply(mubc1, rbc1, "e")
        ls2, lq2 = fc_block("e")
        ln2_out(ls2, lq2, "e")

    nc.compile()
    _BUILD_CACHE[key] = nc
    return nc


def make_in_maps(x, Wq, bq, Wk, bk, Wv, bv, gamma1, beta1, Wfc, bfc, gamma2,
                 beta2):
    x = np.asarray(x, np.float32)
    Wq, Wk, Wv, Wfc = (np.asarray(a, np.float32) for a in (Wq, Wk, Wv, Wfc))
    wq_t = np.ascontiguousarray(
        Wq.reshape(8, 128, 512).transpose(1, 0, 2)).astype(NPBF)
    wk_t = np.ascontiguousarray(
        Wk.reshape(8, 128, 512).transpose(1, 0, 2)).astype(NPBF)
    wv_t = np.ascontiguousarray(
        Wv.reshape(8, 128, 1024).transpose(1, 0, 2)).astype(NPBF)
    wfc_t = np.ascontiguousarray(
        Wfc.reshape(8, 128, 1024).transpose(1, 0, 2)).astype(NPBF)
    bq_t = np.asarray(bq, np.float32).reshape(4, 128).T.copy()
    bk_t = np.asarray(bk, np.float32).reshape(4, 128).T.copy()
    bfc_t = np.asarray(bfc, np.float32).reshape(8, 128).T.copy()
    g1_t = np.asarray(gamma1, np.float32).reshape(8, 128).T.copy()
    b1_t = np.asarray(beta1, np.float32).reshape(8, 128).T.copy()
    g2_t = np.asarray(gamma2, np.float32).reshape(8, 128).T.copy()
    b2_t = np.asarray(beta2, np.float32).reshape(8, 128).T.copy()
    bvt = np.asarray(bv, np.float32).reshape(8, 128).T.copy()
    onescol = np.ones((128, 1), NPBF)
    invdcol = np.full((128, 1), 1.0 / D, NPBF)
    onesrow = np.ones((1, 128), NPBF)
    onesv = np.ones((128, 256), NPBF)
    e64 = np.zeros((1, 256), np.float32)
    e64[0, 0:64] = 1.0        # e64lo: broadcast to partitions 0-63
    e64[0, 192:256] = 1.0     # e64hi: broadcast to partitions 64-127
    e64 = e64.astype(NPBF)

    in_maps = []
    for c in range(8):
        g, r = c // 4, c % 4
        xt = np.ascontiguousarray(
            x[g].T.reshape(8, 128, S).transpose(1, 0, 2)).astype(NPBF)
        in_maps.append({
            "xt": xt,
            "xq": np.ascontiguousarray(xt[:, :, 512 * r:512 * r + 512]),
            "wq": wq_t, "wk": wk_t, "wv": wv_t, "wfc": wfc_t,
            "bq": bq_t, "bk": bk_t, "bfc": bfc_t,
            "g1": g1_t, "b1": b1_t, "g2": g2_t, "b2": b2_t,
            "bvt": bvt, "onescol": onescol, "invdcol": invdcol,
            "onesrow": onesrow,
            "onesv": onesv, "e64": e64,
        })
    return in_maps


def assemble(results):
    out = np.empty((B, S, D), np.float32)
    for c in range(8):
        g, r = c // 4, c % 4
        o = np.asarray(results[c]["out"], np.float32)   # [8, 128, 512]
        for j in range(8):
            out[g, 512 * r:512 * r + 512, 128 * j:128 * j + 128] = o[j].T
    return out


def kernel(**inputs):
    nc = _build()
    in_maps = make_in_maps(**{k: np.asarray(v) for k, v in inputs.items()})
    res = run_bass_kernel_spmd(nc, in_maps, list(range(8)))
    return assemble(res.results)
